# revision 15
# baseline (speedup 1.0000x reference)
"""Trainium2 Bass kernel for nn_Rank_Loss_7438883356888.

Strategy (8 NeuronCores, SPMD, full inputs in / full output out):
  - Anchor-sharded distance mining: core c owns anchors [128c, 128c+128).
    Each core streams the full feature matrix (host-pretiled X^T, bf16)
    and computes its 128 x 4096 squared-distance block via an augmented
    GEMM that produces d2 directly in PSUM (fp32 accumulation):
        d2[a,j] = sum_d (-2 x_a[d]) x_j[d] + sq_a*1 + 1*sq_j
    The sq rows are 3-way split so bf16 quantization of the norms is
    harmless; the remaining bf16 product noise (~1e-3 on distances) is
    negligible for the final loss (the triplet term is ~0.05% of it).
  - Per core, columns are permuted so the same-identity block of its
    anchors always lands at columns [0,128): the Bass program is then
    identical across cores (mining is column-permutation invariant).
  - Pass 1 keeps clamped d2 rows in SBUF (with +BIG on same-id cols) and
    fuses the PSUM->SBUF clamp with the per-block row-min (tensor_scalar
    accum).  Pass 2 mines entirely in d2 space: selection compares d2
    against (gm+0.1)^2 and softmax weights use the linearization
    d ~= gm + (d2-gm2)/(2 gm) (error <= 8e-5), so no elementwise sqrt is
    needed.  Positives use an exact masked softmax on the diag block.
  - Cross-entropy and the side losses are row-sharded 512 rows/core (bf16
    streams, fp32 math); the target logit is fetched with a strided DMA
    gather (cls columns are pre-rolled per core -> core-invariant AP).
  - Each core emits partial scalars; the host combines them.
"""

import os
import numpy as np
import ml_dtypes

import concourse.bass as bass
import concourse.tile as tile
import concourse.mybir as mybir
from concourse import bacc
from concourse.bass_utils import run_bass_kernel_spmd

# ---------------- problem constants (hardcoded per spec) ----------------
N = 4096          # batch rows
D = 2048          # feature dim
P = 1024          # anchors (= N // NUM_INST)
NUM_INST = 4
NCLS = 1024
DSIDE = 1024
NCORES = 8
A = P // NCORES   # 128 anchors per core
R = N // NCORES   # 512 CE/side rows per core
RT = R // 128     # 4 row-tiles per core

MARGIN2 = 0.3
DIVIDE = 3.0
TH_OFF = MARGIN2 / DIVIDE
ALPHA, GAMMA, THETA = 1.0, 0.5, 0.1

BIG2 = 1.0e6      # added to same-id cols (d2 space) to exclude negatives
NEGINF = -1e9     # additive mask for non-positive entries in diag block

W = 1024          # j superblock width (2 PSUM groups of 512)
NSB = N // W      # 4 superblocks
NG = W // 512     # psum groups per superblock
KT = D // 128     # 16 K-tiles of the main GEMM

F32 = mybir.dt.float32
_MMDT_NAME = os.environ.get("BASS_RANK_MMDT", "fp8")
MM_DT = {"bf16": mybir.dt.bfloat16, "f32r": mybir.dt.float32r,
         "f32": mybir.dt.float32, "fp8": mybir.dt.float8e4}[_MMDT_NAME]
# aug rows hold squared norms (~4700) which overflow fp8e4: keep them bf16
_AUGDT_NAME = "bf16" if _MMDT_NAME == "fp8" else _MMDT_NAME
AUG_DT = mybir.dt.bfloat16 if _MMDT_NAME == "fp8" else MM_DT
IO_F32 = os.environ.get("BASS_RANK_F32IO", "0") == "1"
IO_DT = F32 if IO_F32 else mybir.dt.bfloat16

_state: dict = {}


def _build():
    nc = bacc.Bacc("TRN2", target_bir_lowering=False, debug=False,
                   num_devices=NCORES)

    # DRAM I/O (per-core values supplied via in_maps)
    # rhs is host-pretiled: rhs[s*128 + p, t*W + j] = XTperm[t*128+p, s*W+j]
    rhs_h = nc.dram_tensor("rhs", [NSB * 128, KT * W], MM_DT, kind="ExternalInput")
    aug_h = nc.dram_tensor("aug", [6, N], AUG_DT, kind="ExternalInput")
    # lhsT is host-pretiled: lhsT[p, t*A + m] = -2 * XA[m, t*128+p]
    lhsT_h = nc.dram_tensor("lhsT", [128, KT * A], MM_DT, kind="ExternalInput")
    laug_h = nc.dram_tensor("laug", [6, A], AUG_DT, kind="ExternalInput")
    negadd_h = nc.dram_tensor("negadd", [A, A], F32, kind="ExternalInput")
    posadd_h = nc.dram_tensor("posadd", [A, A], F32, kind="ExternalInput")
    cls_h = nc.dram_tensor("cls", [R, NCLS], IO_DT, kind="ExternalInput")
    l2_h = nc.dram_tensor("l2", [R, DSIDE], IO_DT, kind="ExternalInput")
    l3_h = nc.dram_tensor("l3", [R, DSIDE], IO_DT, kind="ExternalInput")
    l4_h = nc.dram_tensor("l4", [R, DSIDE], IO_DT, kind="ExternalInput")
    part_h = nc.dram_tensor("partials", [1, 8], F32, kind="ExternalOutput")

    AX = mybir.AxisListType
    OP = mybir.AluOpType
    AF = mybir.ActivationFunctionType

    with tile.TileContext(nc) as tc:
        with (
            tc.tile_pool(name="pers", bufs=1) as pers,
            tc.tile_pool(name="stream", bufs=2) as stream,
            tc.tile_pool(name="psum", bufs=4, space="PSUM") as psum_pool,
        ):
            # first rhs superblock DMA goes out before everything else
            rhs_tiles = {}
            rhs_tiles[0] = stream.tile([128, KT * W], MM_DT, tag="rhs",
                                       bufs=3, name="rhs_t0")
            for h in range(2):
                KHW = KT * W // 2
                nc.sync.dma_start(rhs_tiles[0][:, h * KHW:(h + 1) * KHW],
                                  rhs_h.ap()[0:128, h * KHW:(h + 1) * KHW])

            lhsT_sb = pers.tile([128, KT * A], MM_DT)
            nc.sync.dma_start(lhsT_sb[:], lhsT_h.ap())
            laug_sb = pers.tile([6, A], AUG_DT)
            nc.sync.dma_start(laug_sb[:], laug_h.ap())
            aug_sb = pers.tile([6, N], AUG_DT)
            nc.sync.dma_start(aug_sb[:], aug_h.ap())
            negadd_sb = pers.tile([A, A], F32)
            nc.sync.dma_start(negadd_sb[:], negadd_h.ap())
            posadd_sb = pers.tile([A, A], F32)
            nc.sync.dma_start(posadd_sb[:], posadd_h.ap())

            dist_all = pers.tile([128, N], F32)   # clamped d2 (masked diag)
            diag_raw = pers.tile([A, A], F32)     # clamped d2 of diag block
            bmin_cols = pers.tile([128, NSB * NG], F32)
            s1cols = pers.tile([128, 4], F32)
            sd2cols = pers.tile([128, 4], F32)
            nmx_cols = pers.tile([128, RT], F32)
            se_cols = pers.tile([128, RT], F32)
            fin = pers.tile([128, 16], F32)
            ones_sb = pers.tile([128, 1], F32)
            gtile = pers.tile([1, R], IO_DT)
            tgsum = pers.tile([1, 1], F32)
            part_sb = pers.tile([1, 8], F32)
            nc.vector.memset(part_sb[:], 0.0)
            nc.vector.memset(fin[:], 0.0)
            nc.vector.memset(ones_sb[:], 1.0)

            CH = 2048

            # batched CE/side input tiles (one DMA each)
            cls_sb = pers.tile([128, RT * NCLS], IO_DT)
            nc.sync.dma_start(
                cls_sb[:].rearrange("p (t c) -> p t c", t=RT),
                cls_h.ap().rearrange("(t p) c -> p t c", p=128))
            l4sb = pers.tile([128, RT * DSIDE], IO_DT)
            nc.sync.dma_start(
                l4sb[:].rearrange("p (t c) -> p t c", t=RT),
                l4_h.ap().rearrange("(t p) c -> p t c", p=128))
            l2sb = pers.tile([128, RT * DSIDE], IO_DT)
            nc.sync.dma_start(
                l2sb[:].rearrange("p (t c) -> p t c", t=RT),
                l2_h.ap().rearrange("(t p) c -> p t c", p=128))
            l3sb = pers.tile([128, RT * DSIDE], IO_DT)
            nc.sync.dma_start(
                l3sb[:].rearrange("p (t c) -> p t c", t=RT),
                l3_h.ap().rearrange("(t p) c -> p t c", p=128))

            def ce_tile(t):
                cls_t = cls_sb[:, t * NCLS:(t + 1) * NCLS]
                nc.vector.tensor_reduce(nmx_cols[:, t:t + 1], cls_t,
                                        AX.X, OP.max, negate=True)
                scrA = stream.tile([128, NCLS], F32, tag="scrA", bufs=4,
                                   name=f"cescr{t}")
                nc.scalar.activation(scrA[:], cls_t, AF.Exp,
                                     bias=nmx_cols[:, t:t + 1], scale=1.0,
                                     accum_out=se_cols[:, t:t + 1])

            def side_tile(t):
                sl = slice(t * DSIDE, (t + 1) * DSIDE)
                d42 = stream.tile([128, DSIDE], F32, tag="scrA", bufs=4,
                                  name=f"d42_{t}")
                nc.vector.tensor_tensor(d42[:], l4sb[:, sl], l2sb[:, sl],
                                        OP.subtract)
                nc.scalar.activation(d42[:], d42[:], AF.Square,
                                     accum_out=fin[:, 5 + t:6 + t])
                d43 = stream.tile([128, DSIDE], F32, tag="scrB", bufs=4,
                                  name=f"d43_{t}")
                nc.vector.tensor_tensor(d43[:], l4sb[:, sl], l3sb[:, sl],
                                        OP.subtract)
                nc.scalar.activation(d43[:], d43[:], AF.Square,
                                     accum_out=fin[:, 9 + t:10 + t])

            # ---------------- distance GEMM + pass 1 (d2 space) ------------
            KH = KT // 2   # k-tiles per DMA half
            for s in range(NSB):
                if s not in rhs_tiles:
                    rhs_tiles[s] = stream.tile([128, KT * W], MM_DT,
                                               tag="rhs", bufs=3,
                                               name=f"rhs_t{s}")
                rhs_t = rhs_tiles[s]
                if s > 0:
                    nchunk = 4 if s == NSB - 1 else 2
                    cw = KT * W // nchunk
                    for h in range(nchunk):
                        nc.sync.dma_start(
                            rhs_t[:, h * cw:(h + 1) * cw],
                            rhs_h.ap()[s * 128:(s + 1) * 128,
                                       h * cw:(h + 1) * cw])
                # k-outer / group-inner: load each weight tile once, run both
                # 512-wide groups on it (second matmul reuses loaded weights)
                pss = [psum_pool.tile([128, 512], F32, tag=f"ps{g}", bufs=2,
                                      name=f"ps{s}_{g}")
                       for g in range(NG)]
                for t in range(KT):
                    for g in range(NG):
                        mm = nc.tensor.matmul(pss[g][:],
                                              lhsT_sb[:, t * A:(t + 1) * A],
                                              rhs_t[:, t * W + g * 512:
                                                    t * W + g * 512 + 512],
                                              start=(t == 0), stop=False)
                        if g > 0:
                            mm.ins.ldweights = False
                for g in range(NG):
                    j0 = s * W + g * 512
                    mm = nc.tensor.matmul(pss[g][:], laug_sb[:],
                                          aug_sb[:, j0:j0 + 512],
                                          start=False, stop=True)
                    if g > 0:
                        mm.ins.ldweights = False

                for g in range(NG):
                    j0 = s * W + g * 512
                    gi = s * NG + g
                    dsl = dist_all[:, j0:j0 + 512]
                    if gi == 0:
                        # diag block lives here; mask before the row-min
                        nc.vector.tensor_scalar(dsl, pss[g][:], 1e-12, None,
                                                OP.max)
                        nc.vector.tensor_copy(diag_raw[:], dist_all[:, 0:A])
                        nc.vector.tensor_tensor(dist_all[:, 0:A],
                                                dist_all[:, 0:A],
                                                negadd_sb[:], OP.add)
                        nc.vector.tensor_reduce(bmin_cols[:, 0:1], dsl,
                                                AX.X, OP.min)
                    else:
                        nc.vector.tensor_scalar(dsl, pss[g][:], 1e-12, None,
                                                OP.max, OP.min,
                                                accum_out=bmin_cols[:, gi:gi + 1])

                # interleaved independent work (keeps engine FIFOs busy)
                if s == 1:
                    ce_tile(0)
                    ce_tile(1)
                if s == 2:
                    ce_tile(2)
                    ce_tile(3)
                    # strided gather of target logits: row r -> cls[r, r//4]
                    nc.sync.dma_start(
                        gtile[:],
                        bass.AP(cls_h, 0, [[NUM_INST * NCLS + 1, R // NUM_INST],
                                           [NCLS, NUM_INST]]))
                    nc.vector.tensor_reduce(tgsum[:], gtile[:], AX.X, OP.add)
                    lncols = fin[:, 1:5]
                    nc.scalar.activation(lncols, se_cols[:], AF.Ln)
                    nc.vector.tensor_tensor(lncols, lncols, nmx_cols[:],
                                            OP.subtract)
                    side_tile(0)
                if s == 3:
                    side_tile(1)
                    side_tile(2)
                    # sqrt table preload + diag conversion off the critical path
                    nc.scalar.activation(diag_raw[:], diag_raw[:], AF.Sqrt)

            side_tile(3)

            # ---------------- mining pass 2 (all in d2 space) ----------------
            negmin2 = pers.tile([128, 1], F32)
            nc.vector.tensor_reduce(negmin2[:], bmin_cols[:], AX.X, OP.min)
            negmin = pers.tile([128, 1], F32)
            nc.scalar.activation(negmin[:], negmin2[:], AF.Sqrt)   # gm

            thresh2 = pers.tile([128, 1], F32)   # (gm + 0.1)^2
            nc.vector.tensor_scalar(thresh2[:], negmin[:], TH_OFF, None, OP.add)
            nc.vector.tensor_tensor(thresh2[:], thresh2[:], thresh2[:], OP.mult)
            gmhalf = pers.tile([128, 1], F32)
            nc.vector.tensor_scalar(gmhalf[:], negmin[:], 0.5, None, OP.mult)
            inv2g = pers.tile([128, 1], F32)
            nc.vector.tensor_scalar(inv2g[:], negmin[:], 2.0, None, OP.mult)
            nc.vector.reciprocal(inv2g[:], inv2g[:])
            inv2gn = pers.tile([128, 1], F32)
            nc.vector.tensor_scalar(inv2gn[:], inv2g[:], -1.0, None, OP.mult)
            # preload the Exp table while the DVE works (input is ready)
            junk11 = pers.tile([1, 1], F32)
            nc.scalar.activation(junk11[:], negmin[0:1, 0:1], AF.Exp)

            cntcols = pers.tile([128, 4], F32)
            tms, ets = [], []
            for q in range(N // CH):
                sl = dist_all[:, q * CH:(q + 1) * CH]
                tm = stream.tile([128, CH], F32, tag="scrB", bufs=4,
                                 name=f"p2m{q}")
                # d2' = d2 + BIG2 * (d2 >= thresh2): excluded -> exp == 0
                nc.vector.tensor_scalar(tm[:], sl, thresh2[:], BIG2,
                                        OP.is_ge, OP.mult)
                nc.vector.tensor_tensor(tm[:], tm[:], sl, OP.add)
                tms.append(tm)
                et = stream.tile([128, CH], F32, tag="scrA", bufs=4,
                                 name=f"p2e{q}")
                # e = exp(gm/2 - d2'/(2 gm)); s1 += sum(e)
                nc.scalar.activation(et[:], tm[:], AF.Exp,
                                     bias=gmhalf[:], scale=inv2gn[:],
                                     accum_out=s1cols[:, q:q + 1])
                ets.append(et)
            for q in range(N // CH):
                sl = dist_all[:, q * CH:(q + 1) * CH]
                # sed2 += sum(e * d2)
                nc.vector.scalar_tensor_tensor(tms[q][:], ets[q][:], 1.0, sl,
                                               OP.mult, OP.mult,
                                               accum_out=sd2cols[:, q:q + 1])

            # positives from the diag block (exact, d space)
            dpos = pers.tile([A, A], F32)
            nc.vector.tensor_tensor(dpos[:], diag_raw[:], posadd_sb[:], OP.add)
            npmax = pers.tile([128, 1], F32)
            nc.vector.tensor_reduce(npmax[:], dpos[:], AX.X, OP.max, negate=True)
            ep = pers.tile([A, A], F32)
            sp1 = pers.tile([128, 1], F32)
            nc.scalar.activation(ep[:], dpos[:], AF.Exp, bias=npmax[:],
                                 scale=1.0, accum_out=sp1[:])
            sp2 = pers.tile([128, 1], F32)
            junk = pers.tile([A, A], F32)
            nc.vector.scalar_tensor_tensor(junk[:], ep[:], 1.0, dpos[:],
                                           OP.mult, OP.mult, accum_out=sp2[:])

            # neg2 = gm/2 + (sum me*d2) / (2 gm * s1) ;  pos2 = sp2 / sp1
            s1 = pers.tile([128, 1], F32)
            nc.vector.tensor_reduce(s1[:], s1cols[:], AX.X, OP.add)
            sd2 = pers.tile([128, 1], F32)
            nc.vector.tensor_reduce(sd2[:], sd2cols[:], AX.X, OP.add)

            r1 = pers.tile([128, 1], F32)
            nc.vector.reciprocal(r1[:], s1[:])
            neg2 = pers.tile([128, 1], F32)
            # neg2 = gm/2 + (sed2 * inv2g) * r1
            nc.vector.scalar_tensor_tensor(neg2[:], sd2[:], inv2g[:], r1[:],
                                           OP.mult, OP.mult)
            nc.vector.tensor_tensor(neg2[:], neg2[:], gmhalf[:], OP.add)
            rp = pers.tile([128, 1], F32)
            nc.vector.reciprocal(rp[:], sp1[:])
            pos2 = pers.tile([128, 1], F32)
            nc.vector.tensor_tensor(pos2[:], sp2[:], rp[:], OP.mult)
            u = fin[:, 0:1]
            # u = relu(margin + (pos2 - neg2))
            nc.vector.scalar_tensor_tensor(u, neg2[:], -1.0, pos2[:],
                                           OP.mult, OP.add)
            nc.vector.tensor_scalar(u, u, MARGIN2, 0.0, OP.add, OP.max)

            # debug columns
            nc.vector.tensor_copy(fin[:, 13:14], negmin[:])
            nc.vector.tensor_copy(fin[:, 14:15], neg2[:])
            nc.vector.tensor_copy(fin[:, 15:16], pos2[:])

            # ---------------- partition reduction via PE ones-matmul --------
            psum_f = psum_pool.tile([1, 16], F32, tag="pf", bufs=1)
            nc.tensor.matmul(psum_f[:], ones_sb[:], fin[:],
                             start=True, stop=True)
            nc.vector.tensor_copy(part_sb[0:1, 0:1], psum_f[0:1, 0:1])
            nc.vector.tensor_reduce(part_sb[0:1, 1:2], psum_f[0:1, 1:5],
                                    AX.X, OP.add)
            nc.vector.tensor_tensor(part_sb[0:1, 1:2], part_sb[0:1, 1:2],
                                    tgsum[:], OP.subtract)
            nc.vector.tensor_reduce(part_sb[0:1, 2:3], psum_f[0:1, 5:9],
                                    AX.X, OP.add)
            nc.vector.tensor_reduce(part_sb[0:1, 3:4], psum_f[0:1, 9:13],
                                    AX.X, OP.add)
            nc.vector.tensor_copy(part_sb[0:1, 4:7], psum_f[0:1, 13:16])
            nc.sync.dma_start(part_h.ap(), part_sb[:])

    nc.compile()
    return nc


# ---------------- host-side data prep ----------------

def _quant(v, dt_name):
    if dt_name == "bf16":
        return v.astype(ml_dtypes.bfloat16)
    if dt_name == "fp8":
        return v.astype(ml_dtypes.float8_e4m3)
    if dt_name == "f32r":
        v32 = v.astype(np.float32)
        return (v32.view(np.uint32) & np.uint32(0xFFFFFC00)).view(np.float32)
    return v.astype(np.float32)


def _split3(v64):
    """3-way split of values so sum of quantized parts ~= exact value."""
    parts = []
    r = v64.astype(np.float64)
    for _ in range(3):
        q = _quant(r, _AUGDT_NAME)
        parts.append(q)
        r = r - q.astype(np.float64)
    return parts


def _mm_np(v):
    return np.ascontiguousarray(_quant(np.asarray(v, np.float32), _MMDT_NAME))


def _io_np(v):
    v = np.asarray(v, np.float32)
    if not IO_F32:
        v = v.astype(ml_dtypes.bfloat16)
    return np.ascontiguousarray(v)


def _prepare_in_maps(cls_fea, l2_side, l3_side, l4_side, input_fea, targets):
    x = np.ascontiguousarray(np.asarray(input_fea, dtype=np.float32))
    t = np.asarray(targets).astype(np.int64)

    # the CE gather + column-roll relies on the PK block fill of targets
    assert np.array_equal(t, np.arange(N) // NUM_INST), \
        "targets do not have the expected arange//NUM_INST structure"

    XT = np.ascontiguousarray(x.T)                       # [D, N] f32
    XTq = _quant(XT, _MMDT_NAME)                         # matmul dtype
    sq64 = (x.astype(np.float64) ** 2).sum(axis=1)       # [N]
    sj = _split3(sq64)
    ones_n = np.ones(N, np.float32)
    aug_base = np.stack([ones_n, ones_n, ones_n, sj[0], sj[1], sj[2]])
    aug_base = _quant(aug_base.astype(np.float32), _AUGDT_NAME)

    cls_fea = np.asarray(cls_fea, dtype=np.float32)
    l2_side = np.asarray(l2_side, dtype=np.float32)
    l3_side = np.asarray(l3_side, dtype=np.float32)
    l4_side = np.asarray(l4_side, dtype=np.float32)

    in_maps = []
    for c in range(NCORES):
        a_sl = slice(A * c, A * c + A)
        lhsT = _quant((-2.0 * x[a_sl]).T.astype(np.float32), _MMDT_NAME)
        lhsT_t = np.ascontiguousarray(
            lhsT.reshape(KT, 128, A).transpose(1, 0, 2).reshape(128, KT * A))
        sa = _split3(sq64[a_sl])
        ones_a = np.ones(A, np.float32)
        laug = np.stack([sa[0].astype(np.float32), sa[1].astype(np.float32),
                         sa[2].astype(np.float32), ones_a, ones_a, ones_a])
        laug = np.ascontiguousarray(_quant(laug, _AUGDT_NAME))

        # column permutation: swap block 0 <-> block c so this core's
        # same-identity columns sit at [0, 128)
        XTp = XTq.copy()
        aug = aug_base.copy()
        if c > 0:
            b = slice(A * c, A * c + A)
            XTp[:, 0:A], XTp[:, b] = XTq[:, b], XTq[:, 0:A]
            aug[:, 0:A], aug[:, b] = aug_base[:, b], aug_base[:, 0:A]
        # pretile: rhs[s*128 + p, t*W + j] = XTp[t*128 + p, s*W + j]
        rhs = np.ascontiguousarray(
            XTp.reshape(KT, 128, NSB, W).transpose(2, 1, 0, 3)
               .reshape(NSB * 128, KT * W))

        a_ids = t[a_sl]
        same = a_ids[:, None] == a_ids[None, :]
        full_counts = (t[None, :] == a_ids[:, None]).sum(axis=1)
        assert (full_counts == same.sum(axis=1)).all(), \
            "targets do not have the expected block structure"
        negadd = np.where(same, BIG2, 0.0).astype(np.float32)
        posadd = np.where(same & ~np.eye(A, dtype=bool), 0.0, NEGINF)
        posadd = posadd.astype(np.float32)

        r_sl = slice(R * c, R * c + R)
        # roll cls columns so the target of local row r is column r//4
        cls_c = _io_np(np.roll(cls_fea[r_sl], -A * c, axis=1))

        in_maps.append({
            "rhs": rhs, "aug": np.ascontiguousarray(aug),
            "lhsT": lhsT_t, "laug": laug,
            "negadd": negadd, "posadd": posadd,
            "cls": cls_c,
            "l2": _io_np(l2_side[r_sl]),
            "l3": _io_np(l3_side[r_sl]),
            "l4": _io_np(l4_side[r_sl]),
        })
    return in_maps


def _combine(results):
    parts = np.stack([results[c]["partials"][0] for c in range(NCORES)])
    trip = parts[:, 0].sum() / P
    xent = parts[:, 1].sum() / N
    loss42 = np.sqrt(parts[:, 2].sum())
    loss43 = np.sqrt(parts[:, 3].sum())
    loss = ALPHA * trip + GAMMA * xent + THETA * (loss42 + loss43)
    return np.float32(loss)


def _get_nc():
    if "nc" not in _state:
        _state["nc"] = _build()
    return _state["nc"]


def _run(in_maps, trace=False, **kw):
    nc = _get_nc()
    return run_bass_kernel_spmd(nc, in_maps, list(range(NCORES)),
                                trace=trace, **kw)


def kernel(cls_fea, l2_side, l3_side, l4_side, input_fea, targets):
    in_maps = _prepare_in_maps(cls_fea, l2_side, l3_side, l4_side,
                               input_fea, targets)
    res = _run(in_maps, trace=False)
    return _combine(res.results)


# revision 16
# speedup vs baseline: 1.0233x; 1.0233x over previous
"""Trainium2 Bass kernel for nn_Rank_Loss_7438883356888.

Strategy (8 NeuronCores, SPMD, full inputs in / full output out):
  - Anchor-sharded distance mining: core c owns anchors [128c, 128c+128).
    Each core streams the full feature matrix (host-pretiled X^T, bf16)
    and computes its 128 x 4096 squared-distance block via an augmented
    GEMM that produces d2 directly in PSUM (fp32 accumulation):
        d2[a,j] = sum_d (-2 x_a[d]) x_j[d] + sq_a*1 + 1*sq_j
    The sq rows are 3-way split so bf16 quantization of the norms is
    harmless; the remaining bf16 product noise (~1e-3 on distances) is
    negligible for the final loss (the triplet term is ~0.05% of it).
  - Per core, columns are permuted so the same-identity block of its
    anchors always lands at columns [0,128): the Bass program is then
    identical across cores (mining is column-permutation invariant).
  - Pass 1 keeps clamped d2 rows in SBUF (with +BIG on same-id cols) and
    fuses the PSUM->SBUF clamp with the per-block row-min (tensor_scalar
    accum).  Pass 2 mines entirely in d2 space: selection compares d2
    against (gm+0.1)^2 and softmax weights use the linearization
    d ~= gm + (d2-gm2)/(2 gm) (error <= 8e-5), so no elementwise sqrt is
    needed.  Positives use an exact masked softmax on the diag block.
  - Cross-entropy and the side losses are row-sharded 512 rows/core (bf16
    streams, fp32 math); the target logit is fetched with a strided DMA
    gather (cls columns are pre-rolled per core -> core-invariant AP).
  - Each core emits partial scalars; the host combines them.
"""

import os
import numpy as np
import ml_dtypes

import concourse.bass as bass
import concourse.tile as tile
import concourse.mybir as mybir
from concourse import bacc
from concourse.bass_utils import run_bass_kernel_spmd

# ---------------- problem constants (hardcoded per spec) ----------------
N = 4096          # batch rows
D = 2048          # feature dim
P = 1024          # anchors (= N // NUM_INST)
NUM_INST = 4
NCLS = 1024
DSIDE = 1024
NCORES = 8
A = P // NCORES   # 128 anchors per core
R = N // NCORES   # 512 CE/side rows per core
RT = R // 128     # 4 row-tiles per core

MARGIN2 = 0.3
DIVIDE = 3.0
TH_OFF = MARGIN2 / DIVIDE
ALPHA, GAMMA, THETA = 1.0, 0.5, 0.1

BIG2 = 1.0e6      # added to same-id cols (d2 space) to exclude negatives
NEGINF = -1e9     # additive mask for non-positive entries in diag block

W = 1024          # j superblock width (2 PSUM groups of 512)
NSB = N // W      # 4 superblocks
NG = W // 512     # psum groups per superblock
KT = D // 128     # 16 K-tiles of the main GEMM

F32 = mybir.dt.float32
_MMDT_NAME = os.environ.get("BASS_RANK_MMDT", "fp8")
MM_DT = {"bf16": mybir.dt.bfloat16, "f32r": mybir.dt.float32r,
         "f32": mybir.dt.float32, "fp8": mybir.dt.float8e4}[_MMDT_NAME]
# aug rows hold squared norms (~4700) which overflow fp8e4: keep them bf16
_AUGDT_NAME = "bf16" if _MMDT_NAME == "fp8" else _MMDT_NAME
AUG_DT = mybir.dt.bfloat16 if _MMDT_NAME == "fp8" else MM_DT
IO_F32 = os.environ.get("BASS_RANK_F32IO", "0") == "1"
IO_DT = F32 if IO_F32 else mybir.dt.bfloat16

_state: dict = {}


def _build():
    nc = bacc.Bacc("TRN2", target_bir_lowering=False, debug=False,
                   num_devices=NCORES)

    # DRAM I/O (per-core values supplied via in_maps)
    # rhs is host-pretiled: rhs[s*128 + p, t*W + j] = XTperm[t*128+p, s*W+j]
    rhs_h = nc.dram_tensor("rhs", [NSB * 128, KT * W], MM_DT, kind="ExternalInput")
    aug_h = nc.dram_tensor("aug", [6, N], AUG_DT, kind="ExternalInput")
    # lhsT is host-pretiled: lhsT[p, t*A + m] = -2 * XA[m, t*128+p]
    lhsT_h = nc.dram_tensor("lhsT", [128, KT * A], MM_DT, kind="ExternalInput")
    laug_h = nc.dram_tensor("laug", [6, A], AUG_DT, kind="ExternalInput")
    negadd_h = nc.dram_tensor("negadd", [A, A], F32, kind="ExternalInput")
    posadd_h = nc.dram_tensor("posadd", [A, A], F32, kind="ExternalInput")
    cls_h = nc.dram_tensor("cls", [R, NCLS], IO_DT, kind="ExternalInput")
    l2_h = nc.dram_tensor("l2", [R, DSIDE], IO_DT, kind="ExternalInput")
    l3_h = nc.dram_tensor("l3", [R, DSIDE], IO_DT, kind="ExternalInput")
    l4_h = nc.dram_tensor("l4", [R, DSIDE], IO_DT, kind="ExternalInput")
    part_h = nc.dram_tensor("partials", [1, 8], F32, kind="ExternalOutput")

    AX = mybir.AxisListType
    OP = mybir.AluOpType
    AF = mybir.ActivationFunctionType

    with tile.TileContext(nc) as tc:
        with (
            tc.tile_pool(name="pers", bufs=1) as pers,
            tc.tile_pool(name="stream", bufs=2) as stream,
            tc.tile_pool(name="psum", bufs=4, space="PSUM") as psum_pool,
        ):
            # first rhs superblock DMA goes out before everything else
            rhs_tiles = {}
            rhs_tiles[0] = stream.tile([128, KT * W], MM_DT, tag="rhs",
                                       bufs=3, name="rhs_t0")
            for h in range(2):
                KHW = KT * W // 2
                nc.sync.dma_start(rhs_tiles[0][:, h * KHW:(h + 1) * KHW],
                                  rhs_h.ap()[0:128, h * KHW:(h + 1) * KHW])

            lhsT_sb = pers.tile([128, KT * A], MM_DT)
            nc.sync.dma_start(lhsT_sb[:], lhsT_h.ap())
            laug_sb = pers.tile([6, A], AUG_DT)
            nc.sync.dma_start(laug_sb[:], laug_h.ap())
            aug_sb = pers.tile([6, N], AUG_DT)
            nc.sync.dma_start(aug_sb[:], aug_h.ap())
            negadd_sb = pers.tile([A, A], F32)
            nc.sync.dma_start(negadd_sb[:], negadd_h.ap())
            posadd_sb = pers.tile([A, A], F32)
            nc.sync.dma_start(posadd_sb[:], posadd_h.ap())

            dist_all = pers.tile([128, N], F32)   # clamped d2 (masked diag)
            diag_raw = pers.tile([A, A], F32)     # clamped d2 of diag block
            bmin_cols = pers.tile([128, NSB * NG], F32)
            s1cols = pers.tile([128, 4], F32)
            sd2cols = pers.tile([128, 4], F32)
            nmx_cols = pers.tile([128, RT], F32)
            se_cols = pers.tile([128, RT], F32)
            fin = pers.tile([128, 16], F32)
            ones_sb = pers.tile([128, 1], F32)
            gtile = pers.tile([1, R], IO_DT)
            tgsum = pers.tile([1, 1], F32)
            part_sb = pers.tile([1, 8], F32)
            nc.vector.memset(part_sb[:], 0.0)
            nc.vector.memset(fin[:], 0.0)
            nc.vector.memset(ones_sb[:], 1.0)

            CH = 1024

            # batched CE/side input tiles (one DMA each)
            cls_sb = pers.tile([128, RT * NCLS], IO_DT)
            nc.sync.dma_start(
                cls_sb[:].rearrange("p (t c) -> p t c", t=RT),
                cls_h.ap().rearrange("(t p) c -> p t c", p=128))
            l4sb = pers.tile([128, RT * DSIDE], IO_DT)
            nc.sync.dma_start(
                l4sb[:].rearrange("p (t c) -> p t c", t=RT),
                l4_h.ap().rearrange("(t p) c -> p t c", p=128))
            l2sb = pers.tile([128, RT * DSIDE], IO_DT)
            nc.sync.dma_start(
                l2sb[:].rearrange("p (t c) -> p t c", t=RT),
                l2_h.ap().rearrange("(t p) c -> p t c", p=128))
            l3sb = pers.tile([128, RT * DSIDE], IO_DT)
            nc.sync.dma_start(
                l3sb[:].rearrange("p (t c) -> p t c", t=RT),
                l3_h.ap().rearrange("(t p) c -> p t c", p=128))

            def ce_tile(t):
                cls_t = cls_sb[:, t * NCLS:(t + 1) * NCLS]
                nc.vector.tensor_reduce(nmx_cols[:, t:t + 1], cls_t,
                                        AX.X, OP.max, negate=True)
                scrA = stream.tile([128, NCLS], F32, tag="scrA", bufs=4,
                                   name=f"cescr{t}")
                nc.scalar.activation(scrA[:], cls_t, AF.Exp,
                                     bias=nmx_cols[:, t:t + 1], scale=1.0,
                                     accum_out=se_cols[:, t:t + 1])

            def side_tile(t):
                sl = slice(t * DSIDE, (t + 1) * DSIDE)
                d42 = stream.tile([128, DSIDE], F32, tag="scrA", bufs=4,
                                  name=f"d42_{t}")
                nc.vector.tensor_tensor(d42[:], l4sb[:, sl], l2sb[:, sl],
                                        OP.subtract)
                nc.scalar.activation(d42[:], d42[:], AF.Square,
                                     accum_out=fin[:, 5 + t:6 + t])
                d43 = stream.tile([128, DSIDE], F32, tag="scrB", bufs=4,
                                  name=f"d43_{t}")
                nc.vector.tensor_tensor(d43[:], l4sb[:, sl], l3sb[:, sl],
                                        OP.subtract)
                nc.scalar.activation(d43[:], d43[:], AF.Square,
                                     accum_out=fin[:, 9 + t:10 + t])

            # ---------------- distance GEMM + pass 1 (d2 space) ------------
            KH = KT // 2   # k-tiles per DMA half
            for s in range(NSB):
                if s not in rhs_tiles:
                    rhs_tiles[s] = stream.tile([128, KT * W], MM_DT,
                                               tag="rhs", bufs=3,
                                               name=f"rhs_t{s}")
                rhs_t = rhs_tiles[s]
                if s > 0:
                    nchunk = 4 if s == NSB - 1 else 2
                    cw = KT * W // nchunk
                    for h in range(nchunk):
                        nc.sync.dma_start(
                            rhs_t[:, h * cw:(h + 1) * cw],
                            rhs_h.ap()[s * 128:(s + 1) * 128,
                                       h * cw:(h + 1) * cw])
                # k-outer / group-inner: load each weight tile once, run both
                # 512-wide groups on it (second matmul reuses loaded weights)
                pss = [psum_pool.tile([128, 512], F32, tag=f"ps{g}", bufs=2,
                                      name=f"ps{s}_{g}")
                       for g in range(NG)]
                for t in range(KT):
                    for g in range(NG):
                        mm = nc.tensor.matmul(pss[g][:],
                                              lhsT_sb[:, t * A:(t + 1) * A],
                                              rhs_t[:, t * W + g * 512:
                                                    t * W + g * 512 + 512],
                                              start=(t == 0), stop=False)
                        if g > 0:
                            mm.ins.ldweights = False
                for g in range(NG):
                    j0 = s * W + g * 512
                    mm = nc.tensor.matmul(pss[g][:], laug_sb[:],
                                          aug_sb[:, j0:j0 + 512],
                                          start=False, stop=True)
                    if g > 0:
                        mm.ins.ldweights = False

                for g in range(NG):
                    j0 = s * W + g * 512
                    gi = s * NG + g
                    dsl = dist_all[:, j0:j0 + 512]
                    if gi == 0:
                        # diag block lives here; mask before the row-min
                        nc.vector.tensor_scalar(dsl, pss[g][:], 1e-12, None,
                                                OP.max)
                        nc.vector.tensor_copy(diag_raw[:], dist_all[:, 0:A])
                        nc.vector.tensor_tensor(dist_all[:, 0:A],
                                                dist_all[:, 0:A],
                                                negadd_sb[:], OP.add)
                        nc.vector.tensor_reduce(bmin_cols[:, 0:1], dsl,
                                                AX.X, OP.min)
                    else:
                        nc.vector.tensor_scalar(dsl, pss[g][:], 1e-12, None,
                                                OP.max, OP.min,
                                                accum_out=bmin_cols[:, gi:gi + 1])

                # interleaved independent work (keeps engine FIFOs busy)
                if s == 1:
                    ce_tile(0)
                    ce_tile(1)
                if s == 2:
                    ce_tile(2)
                    ce_tile(3)
                    # strided gather of target logits: row r -> cls[r, r//4]
                    nc.sync.dma_start(
                        gtile[:],
                        bass.AP(cls_h, 0, [[NUM_INST * NCLS + 1, R // NUM_INST],
                                           [NCLS, NUM_INST]]))
                    nc.vector.tensor_reduce(tgsum[:], gtile[:], AX.X, OP.add)
                    lncols = fin[:, 1:5]
                    nc.scalar.activation(lncols, se_cols[:], AF.Ln)
                    nc.vector.tensor_tensor(lncols, lncols, nmx_cols[:],
                                            OP.subtract)
                    side_tile(0)
                if s == 3:
                    side_tile(1)
                    side_tile(2)
                    # sqrt table preload + diag conversion off the critical path
                    nc.scalar.activation(diag_raw[:], diag_raw[:], AF.Sqrt)

            side_tile(3)

            # ---------------- mining pass 2 (all in d2 space) ----------------
            negmin2 = pers.tile([128, 1], F32)
            nc.vector.tensor_reduce(negmin2[:], bmin_cols[:], AX.X, OP.min)
            negmin = pers.tile([128, 1], F32)
            nc.scalar.activation(negmin[:], negmin2[:], AF.Sqrt)   # gm

            thresh2 = pers.tile([128, 1], F32)   # (gm + 0.1)^2
            nc.vector.tensor_scalar(thresh2[:], negmin[:], TH_OFF, None, OP.add)
            nc.vector.tensor_tensor(thresh2[:], thresh2[:], thresh2[:], OP.mult)
            gmhalf = pers.tile([128, 1], F32)
            nc.vector.tensor_scalar(gmhalf[:], negmin[:], 0.5, None, OP.mult)
            inv2g = pers.tile([128, 1], F32)
            nc.vector.tensor_scalar(inv2g[:], negmin[:], 2.0, None, OP.mult)
            nc.vector.reciprocal(inv2g[:], inv2g[:])
            inv2gn = pers.tile([128, 1], F32)
            nc.vector.tensor_scalar(inv2gn[:], inv2g[:], -1.0, None, OP.mult)
            # preload the Exp table while the DVE works (input is ready)
            junk11 = pers.tile([1, 1], F32)
            nc.scalar.activation(junk11[:], negmin[0:1, 0:1], AF.Exp)

            tms, ets = [], []
            for q in range(N // CH):
                sl = dist_all[:, q * CH:(q + 1) * CH]
                tm = stream.tile([128, CH], F32, tag="scrB", bufs=4,
                                 name=f"p2m{q}")
                # d2' = d2 + BIG2 * (d2 >= thresh2): excluded -> exp == 0
                nc.vector.tensor_scalar(tm[:], sl, thresh2[:], BIG2,
                                        OP.is_ge, OP.mult)
                nc.vector.tensor_tensor(tm[:], tm[:], sl, OP.add)
                tms.append(tm)
                et = stream.tile([128, CH], F32, tag="scrA", bufs=4,
                                 name=f"p2e{q}")
                # e = exp(gm/2 - d2'/(2 gm)); s1 += sum(e)
                nc.scalar.activation(et[:], tm[:], AF.Exp,
                                     bias=gmhalf[:], scale=inv2gn[:],
                                     accum_out=s1cols[:, q:q + 1])
                ets.append(et)
            for q in range(N // CH):
                sl = dist_all[:, q * CH:(q + 1) * CH]
                # sed2 += sum(e * d2)
                nc.vector.scalar_tensor_tensor(tms[q][:], ets[q][:], 1.0, sl,
                                               OP.mult, OP.mult,
                                               accum_out=sd2cols[:, q:q + 1])

            # positives from the diag block (exact, d space)
            dpos = pers.tile([A, A], F32)
            nc.vector.tensor_tensor(dpos[:], diag_raw[:], posadd_sb[:], OP.add)
            npmax = pers.tile([128, 1], F32)
            nc.vector.tensor_reduce(npmax[:], dpos[:], AX.X, OP.max, negate=True)
            ep = pers.tile([A, A], F32)
            sp1 = pers.tile([128, 1], F32)
            nc.scalar.activation(ep[:], dpos[:], AF.Exp, bias=npmax[:],
                                 scale=1.0, accum_out=sp1[:])
            sp2 = pers.tile([128, 1], F32)
            junk = pers.tile([A, A], F32)
            nc.vector.scalar_tensor_tensor(junk[:], ep[:], 1.0, dpos[:],
                                           OP.mult, OP.mult, accum_out=sp2[:])

            # neg2 = gm/2 + (sum me*d2) / (2 gm * s1) ;  pos2 = sp2 / sp1
            s1 = pers.tile([128, 1], F32)
            nc.vector.tensor_reduce(s1[:], s1cols[:], AX.X, OP.add)
            sd2 = pers.tile([128, 1], F32)
            nc.vector.tensor_reduce(sd2[:], sd2cols[:], AX.X, OP.add)

            r1 = pers.tile([128, 1], F32)
            nc.vector.reciprocal(r1[:], s1[:])
            neg2 = pers.tile([128, 1], F32)
            # neg2 = gm/2 + (sed2 * inv2g) * r1
            nc.vector.scalar_tensor_tensor(neg2[:], sd2[:], inv2g[:], r1[:],
                                           OP.mult, OP.mult)
            nc.vector.tensor_tensor(neg2[:], neg2[:], gmhalf[:], OP.add)
            rp = pers.tile([128, 1], F32)
            nc.vector.reciprocal(rp[:], sp1[:])
            pos2 = pers.tile([128, 1], F32)
            nc.vector.tensor_tensor(pos2[:], sp2[:], rp[:], OP.mult)
            u = fin[:, 0:1]
            # u = relu(margin + (pos2 - neg2))
            nc.vector.scalar_tensor_tensor(u, neg2[:], -1.0, pos2[:],
                                           OP.mult, OP.add)
            nc.vector.tensor_scalar(u, u, MARGIN2, 0.0, OP.add, OP.max)

            # debug columns
            nc.vector.tensor_copy(fin[:, 13:14], negmin[:])
            nc.vector.tensor_copy(fin[:, 14:15], neg2[:])
            nc.vector.tensor_copy(fin[:, 15:16], pos2[:])

            # ---------------- partition reduction via PE ones-matmul --------
            psum_f = psum_pool.tile([1, 16], F32, tag="pf", bufs=1)
            nc.tensor.matmul(psum_f[:], ones_sb[:], fin[:],
                             start=True, stop=True)
            nc.vector.tensor_copy(part_sb[0:1, 0:1], psum_f[0:1, 0:1])
            nc.vector.tensor_reduce(part_sb[0:1, 1:2], psum_f[0:1, 1:5],
                                    AX.X, OP.add)
            nc.vector.tensor_tensor(part_sb[0:1, 1:2], part_sb[0:1, 1:2],
                                    tgsum[:], OP.subtract)
            nc.vector.tensor_reduce(part_sb[0:1, 2:3], psum_f[0:1, 5:9],
                                    AX.X, OP.add)
            nc.vector.tensor_reduce(part_sb[0:1, 3:4], psum_f[0:1, 9:13],
                                    AX.X, OP.add)
            nc.vector.tensor_copy(part_sb[0:1, 4:7], psum_f[0:1, 13:16])
            nc.sync.dma_start(part_h.ap(), part_sb[:])

    nc.compile()
    return nc


# ---------------- host-side data prep ----------------

def _quant(v, dt_name):
    if dt_name == "bf16":
        return v.astype(ml_dtypes.bfloat16)
    if dt_name == "fp8":
        return v.astype(ml_dtypes.float8_e4m3)
    if dt_name == "f32r":
        v32 = v.astype(np.float32)
        return (v32.view(np.uint32) & np.uint32(0xFFFFFC00)).view(np.float32)
    return v.astype(np.float32)


def _split3(v64):
    """3-way split of values so sum of quantized parts ~= exact value."""
    parts = []
    r = v64.astype(np.float64)
    for _ in range(3):
        q = _quant(r, _AUGDT_NAME)
        parts.append(q)
        r = r - q.astype(np.float64)
    return parts


def _mm_np(v):
    return np.ascontiguousarray(_quant(np.asarray(v, np.float32), _MMDT_NAME))


def _io_np(v):
    v = np.asarray(v, np.float32)
    if not IO_F32:
        v = v.astype(ml_dtypes.bfloat16)
    return np.ascontiguousarray(v)


def _prepare_in_maps(cls_fea, l2_side, l3_side, l4_side, input_fea, targets):
    x = np.ascontiguousarray(np.asarray(input_fea, dtype=np.float32))
    t = np.asarray(targets).astype(np.int64)

    # the CE gather + column-roll relies on the PK block fill of targets
    assert np.array_equal(t, np.arange(N) // NUM_INST), \
        "targets do not have the expected arange//NUM_INST structure"

    XT = np.ascontiguousarray(x.T)                       # [D, N] f32
    XTq = _quant(XT, _MMDT_NAME)                         # matmul dtype
    sq64 = (x.astype(np.float64) ** 2).sum(axis=1)       # [N]
    sj = _split3(sq64)
    ones_n = np.ones(N, np.float32)
    aug_base = np.stack([ones_n, ones_n, ones_n, sj[0], sj[1], sj[2]])
    aug_base = _quant(aug_base.astype(np.float32), _AUGDT_NAME)

    cls_fea = np.asarray(cls_fea, dtype=np.float32)
    l2_side = np.asarray(l2_side, dtype=np.float32)
    l3_side = np.asarray(l3_side, dtype=np.float32)
    l4_side = np.asarray(l4_side, dtype=np.float32)

    in_maps = []
    for c in range(NCORES):
        a_sl = slice(A * c, A * c + A)
        lhsT = _quant((-2.0 * x[a_sl]).T.astype(np.float32), _MMDT_NAME)
        lhsT_t = np.ascontiguousarray(
            lhsT.reshape(KT, 128, A).transpose(1, 0, 2).reshape(128, KT * A))
        sa = _split3(sq64[a_sl])
        ones_a = np.ones(A, np.float32)
        laug = np.stack([sa[0].astype(np.float32), sa[1].astype(np.float32),
                         sa[2].astype(np.float32), ones_a, ones_a, ones_a])
        laug = np.ascontiguousarray(_quant(laug, _AUGDT_NAME))

        # column permutation: swap block 0 <-> block c so this core's
        # same-identity columns sit at [0, 128)
        XTp = XTq.copy()
        aug = aug_base.copy()
        if c > 0:
            b = slice(A * c, A * c + A)
            XTp[:, 0:A], XTp[:, b] = XTq[:, b], XTq[:, 0:A]
            aug[:, 0:A], aug[:, b] = aug_base[:, b], aug_base[:, 0:A]
        # pretile: rhs[s*128 + p, t*W + j] = XTp[t*128 + p, s*W + j]
        rhs = np.ascontiguousarray(
            XTp.reshape(KT, 128, NSB, W).transpose(2, 1, 0, 3)
               .reshape(NSB * 128, KT * W))

        a_ids = t[a_sl]
        same = a_ids[:, None] == a_ids[None, :]
        full_counts = (t[None, :] == a_ids[:, None]).sum(axis=1)
        assert (full_counts == same.sum(axis=1)).all(), \
            "targets do not have the expected block structure"
        negadd = np.where(same, BIG2, 0.0).astype(np.float32)
        posadd = np.where(same & ~np.eye(A, dtype=bool), 0.0, NEGINF)
        posadd = posadd.astype(np.float32)

        r_sl = slice(R * c, R * c + R)
        # roll cls columns so the target of local row r is column r//4
        cls_c = _io_np(np.roll(cls_fea[r_sl], -A * c, axis=1))

        in_maps.append({
            "rhs": rhs, "aug": np.ascontiguousarray(aug),
            "lhsT": lhsT_t, "laug": laug,
            "negadd": negadd, "posadd": posadd,
            "cls": cls_c,
            "l2": _io_np(l2_side[r_sl]),
            "l3": _io_np(l3_side[r_sl]),
            "l4": _io_np(l4_side[r_sl]),
        })
    return in_maps


def _combine(results):
    parts = np.stack([results[c]["partials"][0] for c in range(NCORES)])
    trip = parts[:, 0].sum() / P
    xent = parts[:, 1].sum() / N
    loss42 = np.sqrt(parts[:, 2].sum())
    loss43 = np.sqrt(parts[:, 3].sum())
    loss = ALPHA * trip + GAMMA * xent + THETA * (loss42 + loss43)
    return np.float32(loss)


def _get_nc():
    if "nc" not in _state:
        _state["nc"] = _build()
    return _state["nc"]


def _run(in_maps, trace=False, **kw):
    nc = _get_nc()
    return run_bass_kernel_spmd(nc, in_maps, list(range(NCORES)),
                                trace=trace, **kw)


def kernel(cls_fea, l2_side, l3_side, l4_side, input_fea, targets):
    in_maps = _prepare_in_maps(cls_fea, l2_side, l3_side, l4_side,
                               input_fea, targets)
    res = _run(in_maps, trace=False)
    return _combine(res.results)
